# revision 1
# baseline (speedup 1.0000x reference)
"""Trainium2 Bass kernel for an AttentionBlock (GroupNorm -> q/k/v 1x1 conv ->
full S x S attention -> proj 1x1 conv -> residual).

Problem shapes: x [4, 512, 64, 64] fp32, S = 4096 tokens, C = 512 channels,
GroupNorm with 32 groups of 16 channels.

Sharding: 8 cores = 4 batches x 2 query-halves. Core c handles batch c//2 and
query rows [half*2048, (half+1)*2048). Each core of a batch-pair redundantly
computes k/v for its batch (cheap vs attention) so no collectives are needed.

Math optimizations baked in:
  * GroupNorm is folded into the q/k/v weights: h = scale_c * x + shift_c with
    per-channel scale/shift derived from group stats, so
    q = (wq * scale) @ x + (bq + wq @ shift), and similarly k, v.
  * k's bias term (bk + wk @ shift) adds a per-query constant to every softmax
    row and cancels exactly -> never computed (bk unused).
  * v's bias adds bv' * sum_j(attn) = bv' to the attention output (softmax rows
    sum to 1), which is then folded into the proj bias:
    bp' = bp + wp @ (bv + wv @ shift).
  * Softmax is computed without max-subtraction (scores are O(10) here, exp is
    safe in fp32), with the denominator accumulated by a ones-vector matmul.

Dtypes: big matmuls run in float32r (full PE speed, ~13-bit mantissa);
attention probabilities and v^T are bf16 (their error is diluted by the
residual connection); everything else fp32.

Layouts per core (partition dim first):
  q  [c=512, i=2048] f32r   (4 tiles [128, 2048])    scores rhs
  k  [c=512, j=4096] f32r   (4 tiles [128, 4096])    scores lhsT (stationary)
  vT [j=4096, c=512] bf16   (1 tile [128, 32, 512])  attn@v lhsT
  scores^T [j, i] so softmax reduction over j uses matmul tricks; attention
  output lands as h [c, i] which feeds proj directly.
"""

import numpy as np
import ml_dtypes

import concourse.bacc as bacc
import concourse.tile as tile
from concourse import mybir
from concourse.bass_utils import run_bass_kernel_spmd

F32 = mybir.dt.float32
F32R = mybir.dt.float32r
BF16 = mybir.dt.bfloat16
AF = mybir.ActivationFunctionType
OP = mybir.AluOpType
AX = mybir.AxisListType

C = 512
S = 4096
B = 4
NCORES = 8
CT = 4          # channel tiles of 128
SBLK = 8        # s-blocks of 512 for k/v/stats
QBLK = 4        # q-blocks of 512 (half = 2048 columns)
IB = 4          # i-blocks of 512 for attention
IBW = 512
JT = 32         # j-tiles of 128
HALF = S // 2
EPS = 1e-5
GELEMS = 16 * S                      # elements per group (16 ch x 4096)
SCL = 1.0 / np.sqrt(np.float32(C))   # softmax scale


def build_nc(reps=1):
    """Build and compile the SPMD single-core program."""
    nc = bacc.Bacc("TRN2", target_bir_lowering=False, debug=False,
                   num_devices=NCORES)

    x_d = nc.dram_tensor("x", [CT, 128, S], F32R, kind="ExternalInput").ap()
    wqt_d = nc.dram_tensor("wqt", [CT, 128, C], F32R, kind="ExternalInput").ap()
    wkt_d = nc.dram_tensor("wkt", [CT, 128, C], F32R, kind="ExternalInput").ap()
    wvt_d = nc.dram_tensor("wvt", [CT, 128, C], F32R, kind="ExternalInput").ap()
    wpt_d = nc.dram_tensor("wpt", [CT, 128, C], F32R, kind="ExternalInput").ap()
    bq_d = nc.dram_tensor("bq", [CT, 128, 1], F32, kind="ExternalInput").ap()
    bv_d = nc.dram_tensor("bv", [CT, 128, 1], F32, kind="ExternalInput").ap()
    bp_d = nc.dram_tensor("bp", [CT, 128, 1], F32, kind="ExternalInput").ap()
    gnw_d = nc.dram_tensor("gnw", [CT, 128, 1], F32, kind="ExternalInput").ap()
    gnb_d = nc.dram_tensor("gnb", [CT, 128, 1], F32, kind="ExternalInput").ap()
    g16_d = nc.dram_tensor("g16", [128, 8], F32, kind="ExternalInput").ap()
    b8_d = nc.dram_tensor("b8", [8, 128], F32, kind="ExternalInput").ap()
    onbf_d = nc.dram_tensor("onbf", [128, 128], F32R, kind="ExternalInput").ap()
    out_d = nc.dram_tensor("out", [CT, 128, HALF], F32, kind="ExternalOutput").ap()

    with tile.TileContext(nc) as tc:
        with tc.tile_pool(name="const", bufs=1) as cpool, \
             tc.tile_pool(name="resident", bufs=1) as rpool:
            # constants loaded once
            g16_t = cpool.tile([128, 8], F32, name="g16t")
            b8_t = cpool.tile([8, 128], F32, name="b8t")
            onbf_t = cpool.tile([128, 128], F32R, name="onbft")
            eps_t = cpool.tile([8, 1], F32, name="epst")
            nc.sync.dma_start(g16_t[:], g16_d[:])
            nc.sync.dma_start(b8_t[:], b8_d[:])
            nc.sync.dma_start(onbf_t[:], onbf_d[:])
            nc.vector.memset(eps_t[:], EPS)
            gnw_t, gnb_t = [], []
            for ci in range(CT):
                gw = cpool.tile([128, 1], F32, name=f"gnw{ci}")
                gb = cpool.tile([128, 1], F32, name=f"gnb{ci}")
                nc.sync.dma_start(gw[:], gnw_d[ci])
                nc.sync.dma_start(gb[:], gnb_d[ci])
                gnw_t.append(gw)
                gnb_t.append(gb)

            for rep in range(reps):
                emit_rep(nc, tc, rpool, rep,
                         x_d, wqt_d, wkt_d, wvt_d, wpt_d,
                         bq_d, bv_d, bp_d,
                         g16_t, b8_t, onbf_t, eps_t, gnw_t, gnb_t,
                         out_d)
    nc.compile()
    return nc


def emit_rep(nc, tc, rpool, rep, x_d, wqt_d, wkt_d, wvt_d, wpt_d,
             bq_d, bv_d, bp_d, g16_t, b8_t, onbf_t, eps_t,
             gnw_t, gnb_t, out_d):
    # ---- resident tensors (slots shared across reps via fixed tags) ----
    k_sb = [rpool.tile([128, S], BF16, name=f"k{ci}_{rep}", tag=f"k{ci}")
            for ci in range(CT)]
    q_sb = [rpool.tile([128, HALF], BF16, name=f"q{ci}_{rep}", tag=f"q{ci}")
            for ci in range(CT)]
    vT_sb = rpool.tile([128, JT, C], BF16, name=f"vT_{rep}", tag="vT")
    wpt_s = [rpool.tile([128, C], F32R, name=f"wp{ci}_{rep}", tag=f"wp{ci}")
             for ci in range(CT)]
    for ci in range(CT):
        nc.sync.dma_start(wpt_s[ci][:], wpt_d[ci])

    with tc.tile_pool(name=f"xblk_{rep}", bufs=2) as xpool, \
         tc.tile_pool(name=f"stat_{rep}", bufs=1) as spool, \
         tc.tile_pool(name=f"sscr_{rep}", bufs=2) as scrpool, \
         tc.tile_pool(name=f"pstat_{rep}", bufs=1, space="PSUM") as pstats:

        # ================= P1: per-channel sum / sumsq over x =================
        sums = spool.tile([128, CT, SBLK], F32, name=f"sums_{rep}", tag="sums")
        sumsq = spool.tile([128, CT, SBLK], F32, name=f"sumsq_{rep}", tag="sumsq")
        for sb in range(SBLK):
            xb = [xpool.tile([128, 512], F32R, name=f"xa{sb}_{ci}_{rep}", tag=f"xb{ci}")
                  for ci in range(CT)]
            for ci in range(CT):
                nc.sync.dma_start(xb[ci][:], x_d[ci, :, sb * 512:(sb + 1) * 512])
            for ci in range(CT):
                xf = xb[ci][:].bitcast(F32)
                nc.vector.reduce_sum(out=sums[:, ci, sb:sb + 1], in_=xf, axis=AX.X)
                sq = scrpool.tile([128, 512], F32, name=f"sq{sb}_{ci}_{rep}", tag="sqscr")
                nc.scalar.activation(out=sq[:], in_=xf, func=AF.Square,
                                     accum_out=sumsq[:, ci, sb:sb + 1])

        # ================= P2: group stats -> per-channel scale/shift =========
        sq2 = spool.tile([128, CT, 2], F32, name=f"sq2_{rep}", tag="sq2")
        for ci in range(CT):
            nc.vector.reduce_sum(out=sq2[:, ci, 0:1], in_=sums[:, ci, :], axis=AX.X)
            nc.vector.reduce_sum(out=sq2[:, ci, 1:2], in_=sumsq[:, ci, :], axis=AX.X)
        gpsum = pstats.tile([8, 8], F32, name=f"gps_{rep}", tag="g")
        for ci in range(CT):
            nc.tensor.matmul(gpsum[:, 2 * ci:2 * ci + 2], g16_t[:], sq2[:, ci, :],
                             start=True, stop=True)
        gp3 = gpsum[:].rearrange("p (c t) -> p c t", t=2)
        packbuf = spool.tile([8, CT, 2], F32, name=f"pack_{rep}", tag="pack")
        ex2 = spool.tile([8, CT], F32, name=f"ex2_{rep}", tag="ex2")
        gm2 = spool.tile([8, CT], F32, name=f"gm2_{rep}", tag="gm2")
        gvar = spool.tile([8, CT], F32, name=f"gvar_{rep}", tag="gvar")
        nc.scalar.mul(out=packbuf[:, :, 1], in_=gp3[:, :, 0], mul=1.0 / GELEMS)
        nc.scalar.mul(out=ex2[:], in_=gp3[:, :, 1], mul=1.0 / GELEMS)
        nc.vector.tensor_mul(gm2[:], packbuf[:, :, 1], packbuf[:, :, 1])
        nc.vector.tensor_sub(gvar[:], ex2[:], gm2[:])
        nc.scalar.activation(out=gvar[:], in_=gvar[:], func=AF.Sqrt,
                             bias=eps_t[:], scale=1.0)
        nc.vector.reciprocal(out=packbuf[:, :, 0], in_=gvar[:])
        scale_t, shift_t = [], []
        for ci in range(CT):
            bca = pstats.tile([128, 2], F32, name=f"bca{ci}_{rep}", tag="bca")
            nc.tensor.matmul(bca[:], b8_t[:], packbuf[:, ci, :], start=True, stop=True)
            sc = spool.tile([128, 1], F32, name=f"scale{ci}_{rep}", tag=f"scale{ci}")
            sh = spool.tile([128, 1], F32, name=f"shift{ci}_{rep}", tag=f"shift{ci}")
            tm = spool.tile([128, 1], F32, name=f"tmpm{ci}_{rep}", tag="tmpm")
            nc.vector.tensor_mul(sc[:], gnw_t[ci][:], bca[:, 0:1])
            nc.vector.tensor_mul(tm[:], bca[:, 1:2], sc[:])
            nc.vector.tensor_sub(sh[:], gnb_t[ci][:], tm[:])
            scale_t.append(sc)
            shift_t.append(sh)

        # ================= P3: fold GN into weights + bias folds ==============
        with tc.tile_pool(name=f"wfold_{rep}", bufs=1) as wfold:
            wq_s, wk_s, wv_s = [], [], []
            for nm, src, lst in (("wq", wqt_d, wq_s), ("wk", wkt_d, wk_s),
                                 ("wv", wvt_d, wv_s)):
                for ci in range(CT):
                    w = wfold.tile([128, C], F32R, name=f"{nm}{ci}_{rep}",
                                   tag=f"{nm}{ci}")
                    nc.sync.dma_start(w[:], src[ci])
                    lst.append(w)
            # bias folds with RAW weights: b' = b + w^T @ shift
            bq_sb, bv_sb = [], []
            for w_s, b_dram, lst, nm in ((wq_s, bq_d, bq_sb, "bq"),
                                         (wv_s, bv_d, bv_sb, "bv")):
                for co in range(CT):
                    pb = pstats.tile([128, 1], F32, name=f"pb{nm}{co}_{rep}", tag="pb")
                    for ci in range(CT):
                        nc.tensor.matmul(
                            pb[:],
                            w_s[ci][:].bitcast(F32)[:, co * 128:(co + 1) * 128],
                            shift_t[ci][:], start=(ci == 0), stop=(ci == CT - 1))
                    braw = spool.tile([128, 1], F32, name=f"{nm}r{co}_{rep}", tag="braw")
                    nc.sync.dma_start(braw[:], b_dram[co])
                    bt = spool.tile([128, 1], F32, name=f"{nm}f{co}_{rep}",
                                    tag=f"{nm}f{co}")
                    nc.vector.tensor_add(bt[:], pb[:], braw[:])
                    lst.append(bt)
            # bp' = bp + wp^T @ bv'
            bp_sb = []
            for co in range(CT):
                pb = pstats.tile([128, 1], F32, name=f"pbbp{co}_{rep}", tag="pb")
                for ci in range(CT):
                    nc.tensor.matmul(
                        pb[:], wpt_s[ci][:].bitcast(F32)[:, co * 128:(co + 1) * 128],
                        bv_sb[ci][:], start=(ci == 0), stop=(ci == CT - 1))
                braw = spool.tile([128, 1], F32, name=f"bpr{co}_{rep}", tag="braw")
                nc.sync.dma_start(braw[:], bp_d[co])
                bt = rpool.tile([128, 1], F32, name=f"bpf{co}_{rep}", tag=f"bpf{co}")
                nc.vector.tensor_add(bt[:], pb[:], braw[:])
                bp_sb.append(bt)
            # scale folds in place (f32 view in, f32r out = rounding write)
            for w_s in (wq_s, wk_s, wv_s):
                for ci in range(CT):
                    nc.vector.tensor_scalar_mul(out=w_s[ci][:],
                                                in0=w_s[ci][:].bitcast(F32),
                                                scalar1=scale_t[ci][:])

            # ================= P4: q / k / vT projections ====================
            with tc.tile_pool(name=f"pd_{rep}", bufs=5, space="PSUM") as pd:
                for sb in range(SBLK):
                    xb = [xpool.tile([128, 512], F32R, name=f"xc{sb}_{ci}_{rep}",
                                     tag=f"xb{ci}") for ci in range(CT)]
                    for ci in range(CT):
                        nc.sync.dma_start(xb[ci][:], x_d[ci, :, sb * 512:(sb + 1) * 512])
                    for co in range(CT):
                        pk = pd.tile([128, 512], F32, name=f"pk{sb}{co}_{rep}", tag="pd")
                        for ci in range(CT):
                            nc.tensor.matmul(pk[:],
                                             wk_s[ci][:, co * 128:(co + 1) * 128],
                                             xb[ci][:], start=(ci == 0),
                                             stop=(ci == CT - 1))
                        nc.vector.tensor_copy(k_sb[co][:, sb * 512:(sb + 1) * 512], pk[:])
                    for js4 in range(4):
                        pv = pd.tile([128, 512], F32, name=f"pv{sb}{js4}_{rep}", tag="pd")
                        for ci in range(CT):
                            nc.tensor.matmul(pv[:],
                                             xb[ci][:, js4 * 128:(js4 + 1) * 128],
                                             wv_s[ci][:], start=(ci == 0),
                                             stop=(ci == CT - 1))
                        nc.scalar.activation(out=vT_sb[:, sb * 4 + js4, :], in_=pv[:],
                                             func=AF.Copy)
                    if sb < QBLK:
                        # columns [0, 2048) are this core's queries (host-permuted)
                        for co in range(CT):
                            pq = pd.tile([128, 512], F32, name=f"pq{sb}{co}_{rep}", tag="pd")
                            for ci in range(CT):
                                nc.tensor.matmul(pq[:],
                                                 wq_s[ci][:, co * 128:(co + 1) * 128],
                                                 xb[ci][:], start=(ci == 0),
                                                 stop=(ci == CT - 1))
                            nc.vector.tensor_scalar(
                                out=q_sb[co][:, sb * 512:(sb + 1) * 512], in0=pq[:],
                                scalar1=bq_sb[co][:], scalar2=None, op0=OP.add)

    # ================= P5: attention + proj + residual =======================
    with tc.tile_pool(name=f"ex_{rep}", bufs=2) as expool, \
         tc.tile_pool(name=f"tsc_{rep}", bufs=1) as tscpool, \
         tc.tile_pool(name=f"hn_{rep}", bufs=1) as hnpool, \
         tc.tile_pool(name=f"eo_{rep}", bufs=3) as eopool, \
         tc.tile_pool(name=f"psc_{rep}", bufs=3, space="PSUM") as psc, \
         tc.tile_pool(name=f"pph_{rep}", bufs=4, space="PSUM") as pph, \
         tc.tile_pool(name=f"psm_{rep}", bufs=1, space="PSUM") as psm:
        for ib in range(IB):
            isl = slice(ib * IBW, (ib + 1) * IBW)
            ex = expool.tile([128, JT, IBW], BF16, name=f"ex{ib}_{rep}", tag="ex")
            for js in range(JT):
                ps_ = psc.tile([128, IBW], F32, name=f"ps{ib}{js}_{rep}", tag="ps")
                for ci in range(CT):
                    nc.tensor.matmul(ps_[:],
                                     k_sb[ci][:, js * 128:(js + 1) * 128],
                                     q_sb[ci][:, isl], start=(ci == 0),
                                     stop=(ci == CT - 1))
                nc.scalar.activation(out=ex[:, js, :], in_=ps_[:], func=AF.Exp,
                                     scale=float(SCL))
            ph = [pph.tile([128, IBW], F32, name=f"ph{ib}{ci}_{rep}", tag="ph")
                  for ci in range(CT)]
            for js in range(JT):
                for ci in range(CT):
                    nc.tensor.matmul(ph[ci][:],
                                     vT_sb[:, js, ci * 128:(ci + 1) * 128],
                                     ex[:, js, :], start=(js == 0),
                                     stop=(js == JT - 1), skip_group_check=True)
            # denominator: bf16 tree over j-tiles (DVE), then exact
            # cross-partition sum via a ones-stationary f32r matmul
            tsc = tscpool.tile([128, 8, IBW], BF16, name=f"tsc{ib}_{rep}", tag="tsc")
            den = hnpool.tile([128, IBW], F32R, name=f"den{ib}_{rep}", tag="den")
            dena = hnpool.tile([128, IBW], F32, name=f"dena{ib}_{rep}", tag="dena")
            denb = hnpool.tile([128, IBW], F32, name=f"denb{ib}_{rep}", tag="denb")
            nc.vector.tensor_add(tsc[:], ex[:, 0:8, :], ex[:, 8:16, :])
            nc.vector.tensor_add(tsc[:, 0:4, :], tsc[:, 0:4, :], tsc[:, 4:8, :])
            nc.vector.tensor_add(tsc[:, 0:2, :], tsc[:, 0:2, :], tsc[:, 2:4, :])
            nc.vector.tensor_add(dena[:], tsc[:, 0, :], tsc[:, 1, :])
            nc.vector.tensor_add(tsc[:], ex[:, 16:24, :], ex[:, 24:32, :])
            nc.vector.tensor_add(tsc[:, 0:4, :], tsc[:, 0:4, :], tsc[:, 4:8, :])
            nc.vector.tensor_add(tsc[:, 0:2, :], tsc[:, 0:2, :], tsc[:, 2:4, :])
            nc.vector.tensor_add(denb[:], tsc[:, 0, :], tsc[:, 1, :])
            nc.vector.tensor_add(den[:], dena[:], denb[:])
            pdn = psm.tile([128, IBW], F32, name=f"pdn{ib}_{rep}", tag="sm")
            nc.tensor.matmul(pdn[:], onbf_t[:], den[:], start=True, stop=True)
            rbc = hnpool.tile([128, IBW], F32, name=f"rbc{ib}_{rep}", tag="rbc")
            nc.vector.reciprocal(out=rbc[:], in_=pdn[:])
            hs = hnpool.tile([128, CT, IBW], F32R, name=f"hs{ib}_{rep}", tag="hs")
            for ci in range(CT):
                nc.vector.tensor_mul(hs[:, ci, :], ph[ci][:], rbc[:])
            for co in range(CT):
                pp = psc.tile([128, IBW], F32, name=f"pp{ib}{co}_{rep}", tag="ps")
                for ci in range(CT):
                    nc.tensor.matmul(pp[:],
                                     wpt_s[ci][:, co * 128:(co + 1) * 128],
                                     hs[:, ci, :], start=(ci == 0),
                                     stop=(ci == CT - 1))
                xr = eopool.tile([128, IBW], F32, name=f"xr{ib}{co}_{rep}", tag="xr")
                nc.sync.dma_start(xr[:], x_d[co, :, isl].bitcast(F32))
                ot = eopool.tile([128, IBW], F32, name=f"ot{ib}{co}_{rep}", tag="ot")
                nc.vector.scalar_tensor_tensor(out=ot[:], in0=pp[:],
                                               scalar=bp_sb[co][:], in1=xr[:],
                                               op0=OP.add, op1=OP.add)
                nc.sync.dma_start(out_d[co, :, isl], ot[:])


# ---------------------------------------------------------------------------
# Host side
# ---------------------------------------------------------------------------
_NC_CACHE = {}


def _get_nc(reps=1):
    if reps not in _NC_CACHE:
        _NC_CACHE[reps] = build_nc(reps)
    return _NC_CACHE[reps]


def make_in_maps(x, gn_w, gn_b, wq, bq, wk, bk, wv, bv, wp, bp):
    xf = np.ascontiguousarray(np.asarray(x, dtype=np.float32)).reshape(B, C, S)
    g16 = np.zeros((128, 8), np.float32)
    g16[np.arange(128), np.arange(128) // 16] = 1.0
    b8 = np.ascontiguousarray(g16.T)
    shared = {
        "wqt": np.ascontiguousarray(np.asarray(wq, np.float32).T).reshape(CT, 128, C),
        "wkt": np.ascontiguousarray(np.asarray(wk, np.float32).T).reshape(CT, 128, C),
        "wvt": np.ascontiguousarray(np.asarray(wv, np.float32).T).reshape(CT, 128, C),
        "wpt": np.ascontiguousarray(np.asarray(wp, np.float32).T).reshape(CT, 128, C),
        "bq": np.asarray(bq, np.float32).reshape(CT, 128, 1),
        "bv": np.asarray(bv, np.float32).reshape(CT, 128, 1),
        "bp": np.asarray(bp, np.float32).reshape(CT, 128, 1),
        "gnw": np.asarray(gn_w, np.float32).reshape(CT, 128, 1),
        "gnb": np.asarray(gn_b, np.float32).reshape(CT, 128, 1),
        "g16": g16,
        "b8": b8,
        "onbf": np.ones((128, 128), np.float32),
    }
    in_maps = []
    for core in range(NCORES):
        b, half = core // 2, core % 2
        xb = xf[b]
        if half == 0:
            xp = xb
        else:
            xp = np.concatenate([xb[:, HALF:], xb[:, :HALF]], axis=1)
        in_maps.append(dict(shared, x=np.ascontiguousarray(xp).reshape(CT, 128, S)))
    return in_maps


def assemble_out(results, H=64, W=64):
    out = np.empty((B, C, S), np.float32)
    for core in range(NCORES):
        b, half = core // 2, core % 2
        out[b][:, half * HALF:(half + 1) * HALF] = \
            results[core]["out"].reshape(C, HALF)
    return out.reshape(B, C, H, W)


def kernel(x, gn_w, gn_b, wq, bq, wk, bk, wv, bv, wp, bp, t1=64, t2=64):
    H, W = int(t1), int(t2)
    nc = _get_nc(1)
    in_maps = make_in_maps(x, gn_w, gn_b, wq, bq, wk, bk, wv, bv, wp, bp)
    res = run_bass_kernel_spmd(nc, in_maps, core_ids=list(range(NCORES)))
    return assemble_out(res.results, H, W)

